# revision 11
# baseline (speedup 1.0000x reference)
"""Trainium2 Bass kernel for nn_ConvBlock (MuLUT-style conv block).

Math (per reference):
  For each of NB=6 branches (3 modes x 2 channels) and 4 rotations:
    - take [x[:,c], prev_x[:,c]], rotate by r, edge-pad by 2 (right/bottom)
    - 3x3 VALID conv (2->64) + bias + relu
    - 4 dense 1x1 layers with channel-concat (MuLUT unit), fp32
    - final 1x1 projection to 8 = OUT_C*SCALE^2 channels (+bias)
    - y := round(tanh(y)*127), pixel-shuffle x2, un-rotate
  Output = sum over branches and rotations / 2.

Sharding: 8 cores = 8 (rotation r, batch b) pairs; each core computes all 6
branches for its rotated image. Host does rotation/padding/im2col prep and the
B6-bias/tanh/round/pixel-shuffle/un-rotate/sum epilogue (elementwise or
permutation; fp32-exact — the branch sums are small integers).

Device kernel (per core, fully fp32r on the PE):
  - All matmuls use the uniform (128, 64) tile mode at tile_position (0,0):
    K=128 (zero-padded where the real contraction is smaller), M=64, N=512.
    fp32r streams 1 row/cycle at N>=256 but cannot use array col/row tiling,
    so psum outputs always land at partitions 0-63.
  - Layer activations are stacked in pairs for K=128 rhs tiles:
    H01 = [h0 @ partitions 0-63 ; h1 @ 64-127], H23 likewise, H4 = [h4; 0].
    The relu (bias+max via one tensor_scalar) reads psum[0:64] and writes
    either the lower or (partition-shifted) upper half of the stacked tile.
  - Zero-padded lhsT rows make the "not yet written / stale" upper halves
    harmless; a one-time gpsimd memset of the pool slots guarantees the very
    first tenants are finite.
  - w6 psum [8, 512] is DMA'd straight from PSUM to DRAM (B6 added on host).
"""

import os
import sys

import numpy as np

if "/opt/trn_rl_repo" not in sys.path:
    sys.path.insert(0, "/opt/trn_rl_repo")

IN_C, OUT_C, SCALE, S, NF = 2, 2, 2, 3, 64
MODES = 3
NB = IN_C * MODES
PAD = S - 1
B, H = 2, 64
NPIX = H * H            # 4096 pixels per (batch, rotation) image
NCH = 8                 # chunks of 512 pixels
CW = 512                # chunk width (psum bank, fp32)
N_CORES = 8
HBUFS = 6

_BASS_CACHE = {}


def _build_bass():
    import concourse.bass as bass  # noqa: F401
    import concourse.mybir as mybir
    from concourse import bacc
    from concourse.tile import TileContext

    f32 = mybir.dt.float32
    f32r = mybir.dt.float32r
    Alu = mybir.AluOpType
    Act = mybir.ActivationFunctionType

    nc = bacc.Bacc(
        "TRN2",
        target_bir_lowering=False,
        debug=False,
        enable_asserts=False,
        num_devices=N_CORES,
    )

    xcol_d = nc.dram_tensor("xcol", [2, 18, NPIX], f32r, kind="ExternalInput")
    w1_d = nc.dram_tensor("w1", [NB, 128, 64], f32r, kind="ExternalInput")
    wd_d = nc.dram_tensor("wd", [NB, 9, 128, 64], f32r, kind="ExternalInput")
    bv_d = nc.dram_tensor("bvec", [64, NB * 5], f32, kind="ExternalInput")
    yout_d = nc.dram_tensor("yout", [NB, 8, NPIX], f32, kind="ExternalOutput")

    with TileContext(nc) as tc:
        with (
            tc.tile_pool(name="const", bufs=1) as cpool,
            tc.tile_pool(name="hpool", bufs=HBUFS) as hpool,
            tc.tile_pool(name="psum", bufs=8, space="PSUM") as ppool,
        ):
            xcol2 = cpool.tile([128, NPIX], f32r, name="xcol2")
            wconv = cpool.tile([128, NB * 64], f32r, name="wconv")
            wdense = cpool.tile([128, NB * 9 * 64], f32r, name="wdense")
            bvec = cpool.tile([64, NB * 5], f32, name="bvec")

            # rows 0-35: im2col taps; rows 36-127: zeros (K padded to 128).
            # memset needs a 32-aligned start partition; the DMA then
            # overwrites rows 32-35 (WAW dep keeps the order).
            nc.gpsimd.memset(xcol2[32:64, :].bitcast(f32), 0.0)
            nc.gpsimd.memset(xcol2[64:128, :].bitcast(f32), 0.0)
            nc.sync.dma_start(
                out=xcol2[0:36, :], in_=xcol_d.ap().rearrange("c k n -> (c k) n")
            )
            nc.sync.dma_start(
                out=wconv[:, :].rearrange("k (b m) -> k b m", b=NB),
                in_=w1_d.ap().rearrange("b k m -> k b m"),
            )
            nc.sync.dma_start(
                out=wdense[:, :].rearrange("k (b t m) -> k b t m", b=NB, t=9),
                in_=wd_d.ap().rearrange("b t k m -> k b t m"),
            )
            nc.sync.dma_start(out=bvec[:, :], in_=bv_d.ap())

            # Fixed per-lane stacked activation tiles. Lanes give cross-chunk
            # pipelining (like pool bufs) but reuse the SAME tensors, so the
            # reads of not-yet-written upper halves (nullified by zero lhsT
            # rows) target regions this tensor has written before — satisfying
            # the race checker. One memset makes the first tenants finite.
            lanes = []
            for i in range(HBUFS):
                l01 = cpool.tile([128, CW], f32r, name=f"h01L{i}")
                l23 = cpool.tile([128, CW], f32r, name=f"h23L{i}")
                l4 = cpool.tile([128, CW], f32r, name=f"h4L{i}")
                for t in (l01, l23, l4):
                    nc.gpsimd.memset(t[64:128, :].bitcast(f32), 0.0)
                lanes.append((l01, l23, l4))

            def wtile(b, t):
                c0 = (b * 9 + t) * 64
                return wdense[:, c0 : c0 + 64]

            # dense layer -> (k-tile indices, rhs stack index list)
            # stacks: 0 = H01, 1 = H23, 2 = H4
            LAYERS = [
                ([0], [0]),            # w2: K over (h0, 0)
                ([1], [0]),            # w3: K over (h0, h1)
                ([2, 3], [0, 1]),      # w4: (h0,h1) + (h2, 0)
                ([4, 5], [0, 1]),      # w5: (h0,h1) + (h2,h3)
                ([6, 7, 8], [0, 1, 2]),  # w6: + (h4, 0)
            ]

            for br in range(NB):
                for n in range(NCH):
                    sl = slice(n * CW, (n + 1) * CW)

                    ps0 = ppool.tile([128, CW], f32, name=f"ps0_{br}_{n}", tag="ps")
                    nc.tensor.matmul(
                        ps0[0:64, :],
                        lhsT=wconv[:, br * 64 : (br + 1) * 64],
                        rhs=xcol2[:, sl],
                        start=True,
                        stop=True,
                    )
                    h01, h23, h4 = lanes[(br * NCH + n) % HBUFS]
                    stacks = [h01, h23, h4]
                    # relu targets for h0..h4: (stack, partition offset)
                    rtarget = [(h01, 0), (h01, 64), (h23, 0), (h23, 64), (h4, 0)]

                    def relu(lidx, psum_tile):
                        tile_, off = rtarget[lidx]
                        out_ap = tile_[off : off + 64, :]
                        bias_ap = bvec[:, br * 5 + lidx : br * 5 + lidx + 1]
                        if lidx in (0, 2, 4):
                            nc.vector.tensor_scalar(
                                out=out_ap,
                                in0=psum_tile[0:64, :],
                                scalar1=bias_ap,
                                scalar2=0.0,
                                op0=Alu.add,
                                op1=Alu.max,
                            )
                        else:
                            nc.scalar.activation(
                                out_ap,
                                psum_tile[0:64, :],
                                Act.Relu,
                                bias=bias_ap,
                                scale=1.0,
                            )

                    relu(0, ps0)

                    for l, (ktiles, rhss) in enumerate(LAYERS):
                        psl = ppool.tile(
                            [128, CW], f32, name=f"ps{l + 1}_{br}_{n}", tag="ps"
                        )
                        mwidth = 64  # zero-padded M for w6 keeps (128,64) mode
                        for i, (t, si) in enumerate(zip(ktiles, rhss)):
                            nc.tensor.matmul(
                                psl[0:mwidth, :],
                                lhsT=wtile(br, t),
                                rhs=stacks[si][:, :],
                                start=i == 0,
                                stop=i == len(ktiles) - 1,
                            )
                        if l < 4:
                            relu(l + 1, psl)
                        else:
                            yt = hpool.tile([8, CW], f32, name=f"y_{br}_{n}", tag="y")
                            if (br * NCH + n) % 2 == 0:
                                nc.vector.tensor_copy(yt[:, :], psl[0:8, :])
                            else:
                                nc.scalar.copy(yt[:, :], psl[0:8, :])
                            nc.sync.dma_start(out=yout_d.ap()[br, :, sl], in_=yt[:, :])

    nc.compile()
    return nc


def _get_bass():
    if "nc" not in _BASS_CACHE:
        _BASS_CACHE["nc"] = _build_bass()
    return _BASS_CACHE["nc"]


def _im2col_core(x, prev_x, r, b):
    """Per-core input: [2 (channel c), 18 (ci*9+dy*3+dx), 4096] fp32."""
    cols = np.empty((2, 18, NPIX), np.float32)
    for c in range(2):
        xc = np.stack([x[b, c], prev_x[b, c]])  # [2, 64, 64]
        rot = np.rot90(xc, k=r, axes=(1, 2))
        padd = np.pad(rot, ((0, 0), (0, PAD), (0, PAD)), mode="edge")  # [2,66,66]
        for ci in range(2):
            for dy in range(3):
                for dx in range(3):
                    cols[c, ci * 9 + dy * 3 + dx] = padd[
                        ci, dy : dy + H, dx : dx + H
                    ].reshape(-1)
    return cols


def _prep_weights(W1, B1, W2, B2, W3, B3, W4, B4, W5, B5, W6, B6):
    w1 = np.zeros((NB, 128, 64), np.float32)
    for br in range(NB):
        c = br // MODES
        base = c * 18
        # W1[br]: [64(out), 2(ci), 3, 3] -> rows ci*9+dy*3+dx, cols out
        w1[br, base : base + 18, :] = W1[br].transpose(1, 2, 3, 0).reshape(18, 64)

    wd = np.zeros((NB, 9, 128, 64), np.float32)
    for br in range(NB):
        t = 0
        for W in (W2, W3, W4, W5, W6):
            wt = np.ascontiguousarray(W[br].T)  # [K, M]
            K, M = wt.shape
            for k0 in range(0, K, 128):
                rows = min(128, K - k0)
                wd[br, t, :rows, :M] = wt[k0 : k0 + rows]
                t += 1
        assert t == 9

    bvec = np.zeros((64, NB * 5), np.float32)
    for br in range(NB):
        for j, bb in enumerate((B1, B2, B3, B4, B5)):
            bvec[:, br * 5 + j] = bb[br]
    return w1, wd, bvec


def _postprocess(y_per_core, B6):
    """y_per_core[core] = yout [6, 8, 4096] (pre-B6); core = r*2 + b."""
    out = np.zeros((B, OUT_C, SCALE * H, SCALE * H), np.float32)
    for core in range(N_CORES):
        r, b = core // B, core % B
        y6 = y_per_core[core] + B6[:, :, None]  # [6, 8, 4096]
        y6 = y6.reshape(NB, 8, H, H)
        z = np.round(np.tanh(y6) * np.float32(127.0))
        # pixel shuffle x2: [6, 8, 64, 64] -> [6, 2, 128, 128]
        zz = (
            z.reshape(NB, OUT_C, SCALE, SCALE, H, H)
            .transpose(0, 1, 4, 2, 5, 3)
            .reshape(NB, OUT_C, SCALE * H, SCALE * H)
        )
        un = np.rot90(zz, k=(4 - r) % 4, axes=(2, 3))
        out[b] += un.sum(axis=0, dtype=np.float32)
    out /= np.float32(IN_C)
    return out


def kernel(x, prev_x, W1, B1, W2, B2, W3, B3, W4, B4, W5, B5, W6, B6,
           _trace=False):
    from concourse.bass_utils import run_bass_kernel_spmd

    args = [np.ascontiguousarray(np.asarray(a), dtype=np.float32) for a in
            (x, prev_x, W1, B1, W2, B2, W3, B3, W4, B4, W5, B5, W6, B6)]
    x, prev_x, W1, B1, W2, B2, W3, B3, W4, B4, W5, B5, W6, B6 = args

    w1, wd, bvec = _prep_weights(W1, B1, W2, B2, W3, B3, W4, B4, W5, B5, W6, B6)

    in_maps = []
    for core in range(N_CORES):
        r, b = core // B, core % B
        in_maps.append(
            {
                "xcol": _im2col_core(x, prev_x, r, b),
                "w1": w1,
                "wd": wd,
                "bvec": bvec,
            }
        )

    nc = _get_bass()
    res = run_bass_kernel_spmd(
        nc, in_maps, core_ids=list(range(N_CORES)), trace=_trace
    )
    _BASS_CACHE["last_results"] = res
    return _postprocess([res.results[c]["yout"] for c in range(N_CORES)], B6)
